# revision 1
# baseline (speedup 1.0000x reference)
"""AxialAttention Trainium2 Bass kernel.

Problem: q,k,v of shape (4, 8, 16, 32, 32, 64) = (b, heads, t, h, w, d),
attention along the h axis (axis 3), softmax over keys, out same shape.

Decomposition: the computation is 512 independent "slabs" (b, heads, t),
each a batch of w=32 independent length-32 attention problems with head
dim 64.  We shard 64 slabs per NeuronCore (8 cores), and process slabs in
"quads" (4 slabs = 128 partitions).

Per quad:
  - Load Q,K natural tiles [128=(s,h), 2048=(w,d)] with fp32->bf16 cast DMA.
  - DVE 32x32 stream-transpose -> QT,KT [128=(s,dlo), (w,db,h)].
  - Scores: per (w, db, s) a K=32 matmul at tile_position (32s, 0),
    accumulating db=0,1; outputs scores^T [k, q] in per-s PSUM banks.
  - exp on ScalarE (scale = 1/sqrt(64)) -> E_s bf16.
  - PV: per (w, s) a K=32 matmul lhsT=E block, rhs=[V | 1] (V augmented
    with a ones column so the softmax denominator falls out of the same
    matmul), tile_position (0, 32s) -> psum [(s,q), (w4, 65)].
  - reciprocal of denominators, copy unnormalized out, one broadcasted
    tensor_mul to normalize, store [128=(s,h), (w,d)] contiguous.
"""

import os
import sys
import numpy as np

for _p in ("/root/.axon_site/_ro/trn_rl_repo", "/opt/trn_rl_repo"):
    if os.path.isdir(_p) and _p not in sys.path:
        sys.path.append(_p)

B, NH, T, H, W, D = 4, 8, 16, 32, 32, 64
N_CORES = 8
NSLAB = B * NH * T  # 512
NSLAB_CORE = NSLAB // N_CORES  # 64
NQUAD = NSLAB_CORE // 4  # 16
VST = 80  # padded per-(s,w) V row: 64 d + 1 one + 15 pad (32B aligned)

_CACHED_NC = None


def _build_nc(n_slabs):
    import concourse.bacc as bacc
    import concourse.mybir as mybir
    from concourse import tile

    dt = mybir.dt
    nq = n_slabs // 4

    nc = bacc.Bacc("TRN2", target_bir_lowering=False, debug=False,
                   num_devices=N_CORES)
    q_in = nc.dram_tensor("q_in", [n_slabs, H, W, D], dt.bfloat16,
                          kind="ExternalInput").ap()
    k_in = nc.dram_tensor("k_in", [n_slabs, H, W, D], dt.bfloat16,
                          kind="ExternalInput").ap()
    v_in = nc.dram_tensor("v_in", [n_slabs, H, W, D], dt.bfloat16,
                          kind="ExternalInput").ap()
    o_out = nc.dram_tensor("o_out", [n_slabs, H, W, D], dt.float32,
                           kind="ExternalOutput").ap()

    scale = 1.0 / float(np.sqrt(D))

    with tile.TileContext(nc) as tc:
        with tc.tile_pool(name="io", bufs=3) as io_pool, \
             tc.tile_pool(name="tp", bufs=3) as tp_pool, \
             tc.tile_pool(name="vv", bufs=2) as v_pool, \
             tc.tile_pool(name="ee", bufs=3) as e_pool, \
             tc.tile_pool(name="oo", bufs=2) as o_pool, \
             tc.tile_pool(name="rr", bufs=2) as r_pool, \
             tc.tile_pool(name="ps_sc", bufs=1, space="PSUM") as ps_sc, \
             tc.tile_pool(name="ps_pv", bufs=1, space="PSUM") as ps_pv:

            quad_state = {}

            def emit_loads(g):
                s0 = 4 * g
                Q4 = io_pool.tile([128, W * D], dt.bfloat16, name="Q4")
                K4 = io_pool.tile([128, W * D], dt.bfloat16, name="K4")
                V4 = v_pool.tile([32, 4, W, VST], dt.bfloat16, name="V4")
                nc.sync.dma_start(
                    out=Q4[:, :],
                    in_=q_in[s0:s0 + 4].rearrange("s h w d -> (s h) (w d)"))
                nc.sync.dma_start(
                    out=K4[:, :],
                    in_=k_in[s0:s0 + 4].rearrange("s h w d -> (s h) (w d)"))
                for s in range(4):
                    nc.gpsimd.dma_start(
                        out=V4[:, s, :, 0:D],
                        in_=v_in[s0 + s])
                nc.vector.memset(V4[:, :, :, D:D + 1], 1.0)
                QT = tp_pool.tile([128, W * D], dt.bfloat16, name="QT")
                KT = tp_pool.tile([128, W * D], dt.bfloat16, name="KT")
                nc.vector.transpose(QT[:, :], Q4[:, :])
                nc.vector.transpose(KT[:, :], K4[:, :])
                out_sb = o_pool.tile([128, W, D], dt.float32, name="out_sb")
                R = r_pool.tile([128, W], dt.float32, name="R")
                quad_state[g] = dict(QT=QT, KT=KT, V4=V4, out_sb=out_sb, R=R)

            def emit_scores(g, chunk):
                qs = quad_state[g]
                QT, KT = qs["QT"], qs["KT"]
                w0 = 16 * chunk
                psS = [ps_sc.tile([32, 512], dt.float32, name=f"psS{s}")
                       for s in range(4)]
                Es = []
                # s-outer: each s-tile finishes early so its exp overlaps
                # the next s-tile's matmuls.
                for s in range(4):
                    for wl in range(16):
                        w = w0 + wl
                        for db in range(2):
                            c = (2 * w + db) * 32
                            nc.tensor.matmul(
                                psS[s][0:32, 32 * wl:32 * wl + 32],
                                lhsT=KT[32 * s:32 * s + 32, c:c + 32],
                                rhs=QT[32 * s:32 * s + 32, c:c + 32],
                                start=(db == 0), stop=(db == 1),
                                tile_position=(32 * s, 0))
                    E = e_pool.tile([32, 512], dt.bfloat16, name=f"E{s}")
                    nc.scalar.activation(
                        E[:, :], psS[s][:, :],
                        mybir.ActivationFunctionType.Exp, scale=scale)
                    Es.append(E)
                return Es

            def emit_pv(g, chunk, Es):
                qs = quad_state[g]
                V4, out_sb, R = qs["V4"], qs["out_sb"], qs["R"]
                w0 = 16 * chunk
                psPVs = [ps_pv.tile([128, 4, D + 1], dt.float32,
                                    name=f"psPV{i_}") for i_ in range(4)]
                for s in range(4):
                    for wl in range(16):
                        w = w0 + wl
                        psPV = psPVs[wl // 4]
                        wl4 = wl % 4
                        nc.tensor.matmul(
                            psPV[32 * s:32 * s + 32, wl4:wl4 + 1, 0:D + 1],
                            lhsT=Es[s][:, 32 * wl:32 * wl + 32],
                            rhs=V4[0:32, s, w, 0:D + 1],
                            start=True, stop=True,
                            tile_position=(0, 32 * s))
                for grp in range(4):
                    psPV = psPVs[grp]
                    nc.vector.reciprocal(
                        R[:, w0 + 4 * grp:w0 + 4 * grp + 4],
                        psPV[:, :, D])
                    nc.scalar.activation(
                        out_sb[:, w0 + 4 * grp:w0 + 4 * grp + 4, :],
                        psPV[:, :, 0:D],
                        mybir.ActivationFunctionType.Copy)

            def emit_finish(g):
                qs = quad_state.pop(g)
                out_sb, R = qs["out_sb"], qs["R"]
                s0 = 4 * g
                nc.vector.tensor_mul(
                    out_sb[:, :, :], out_sb[:, :, :],
                    R[:, :, None].broadcast_to([128, W, D]))
                nc.sync.dma_start(
                    out=o_out[s0:s0 + 4].rearrange("s h w d -> (s h) w d"),
                    in_=out_sb[:, :, :])

            # Software pipeline: PV of chunk t is emitted after the scores
            # of chunk t+1, so the PE queue always has runnable matmuls
            # while exp/copy of the previous chunk drain on ScalarE.
            emit_loads(0)
            pending = None  # (g, chunk, Es)
            for t in range(2 * nq):
                g, chunk = divmod(t, 2)
                if chunk == 0 and g + 1 < nq:
                    emit_loads(g + 1)
                Es = emit_scores(g, chunk)
                if pending is not None:
                    pg, pc, pEs = pending
                    emit_pv(pg, pc, pEs)
                    if pc == 1:
                        emit_finish(pg)
                pending = (g, chunk, Es)
            pg, pc, pEs = pending
            emit_pv(pg, pc, pEs)
            emit_finish(pg)
    nc.compile()
    return nc


def _get_nc():
    global _CACHED_NC
    if _CACHED_NC is None:
        _CACHED_NC = _build_nc(NSLAB_CORE)
    return _CACHED_NC


def kernel(q, k, v, decode_step=0, decode_idx=0, _trace=False):
    from concourse.bass_utils import run_bass_kernel_spmd

    import ml_dtypes
    bf16 = ml_dtypes.bfloat16
    q = np.asarray(q, dtype=np.float32).reshape(NSLAB, H, W, D).astype(bf16)
    k = np.asarray(k, dtype=np.float32).reshape(NSLAB, H, W, D).astype(bf16)
    v = np.asarray(v, dtype=np.float32).reshape(NSLAB, H, W, D).astype(bf16)

    nc = _get_nc()
    in_maps = []
    for c in range(N_CORES):
        sl = slice(c * NSLAB_CORE, (c + 1) * NSLAB_CORE)
        in_maps.append({
            "q_in": np.ascontiguousarray(q[sl]),
            "k_in": np.ascontiguousarray(k[sl]),
            "v_in": np.ascontiguousarray(v[sl]),
        })
    res = run_bass_kernel_spmd(nc, in_maps, core_ids=list(range(N_CORES)),
                               trace=_trace)
    out = np.concatenate([r["o_out"] for r in res.results], axis=0)
    out = out.reshape(B, NH, T, H, W, D)
    if _trace:
        return out, res
    return out


if __name__ == "__main__":
    rng = np.random.default_rng(0)
    shape = (B, NH, T, H, W, D)
    q = rng.standard_normal(shape, dtype=np.float32)
    k = rng.standard_normal(shape, dtype=np.float32)
    v = rng.standard_normal(shape, dtype=np.float32)
    out = kernel(q, k, v)
    print("kernel ran, out shape", out.shape)



# revision 3
# speedup vs baseline: 1.1634x; 1.1634x over previous
"""AxialAttention Trainium2 Bass kernel (v2).

Problem: q,k,v of shape (4, 8, 16, 32, 32, 64) = (b, heads, t, h, w, d),
attention along the h axis (axis 3), softmax over keys, out same shape.

The computation is 512 independent "slabs" (b, heads, t), each a batch of
w=32 independent length-32 attention problems with head dim 64.  64 slabs
per NeuronCore (8 cores), processed in "quads" (4 slabs = 128 partitions).

Design notes (all matmul cost on the PE is ~ output-free-size columns, so
outputs are kept 128 partitions wide and as narrow as possible in free):

  - Host pre-transposes Q and K to d-major layout, so no on-chip
    transposes are needed and every DMA is fully contiguous.
  - Scores: per (slab j, w) one K=64 matmul; the four slabs of a quad are
    packed as two "pair" tiles [128=(jj,d64), ...] and placed at PE
    quadrants (64*jj, 32*j), producing psS [128=(j,k), (w,q)] in PSUM.
  - exp on ScalarE over [128, 512] tiles (scale = 1/sqrt(64)).
  - Softmax denominator: one N=512 matmul per 16-w group with a constant
    block-diagonal ones matrix as weights: psD[(j,r),(w,q)] = sum_k
    E[(j,k),(w,q)].  Since every partition of a band carries the same
    value, RD = 1/psD aligns with E partition-for-partition, and
    E2 = E * RD normalizes E before PV (DVE).
  - PV: per (slab j, w) one K=32 matmul lhsT=E2 block, rhs=V natural
    [128=(j,k), (w,d)], at PE quadrant (32j, 32j) -> psPV [128=(j,q), d].
  - psPV copied (and cast) to bf16 out_sb, stored contiguously; host
    casts back to fp32.
"""

import os
import sys
import numpy as np

for _p in ("/root/.axon_site/_ro/trn_rl_repo", "/opt/trn_rl_repo"):
    if os.path.isdir(_p) and _p not in sys.path:
        sys.path.append(_p)

B, NH, T, H, W, D = 4, 8, 16, 32, 32, 64
N_CORES = 8
NSLAB = B * NH * T  # 512
NSLAB_CORE = NSLAB // N_CORES  # 64
NQUAD = NSLAB_CORE // 4  # 16
NGRP = 2 * NQUAD  # 16-w score/exp/pv groups per core

_CACHED_NC = None


def _build_nc():
    import concourse.bacc as bacc
    import concourse.mybir as mybir
    from concourse import tile

    dt = mybir.dt

    nc = bacc.Bacc("TRN2", target_bir_lowering=False, debug=False,
                   num_devices=N_CORES)
    # (quad, pair, (jj,d64), (w,q))
    qt_in = nc.dram_tensor("qt_in", [NQUAD, 2, 128, W * 32], dt.bfloat16,
                           kind="ExternalInput").ap()
    kt_in = nc.dram_tensor("kt_in", [NQUAD, 2, 128, W * 32], dt.bfloat16,
                           kind="ExternalInput").ap()
    # (quad, (j,k=h), (w,d))
    v_in = nc.dram_tensor("v_in", [NQUAD, 128, W * D], dt.bfloat16,
                          kind="ExternalInput").ap()
    # (quad, (j,q=h), (w,d))
    o_out = nc.dram_tensor("o_out", [NQUAD, 128, W * D], dt.bfloat16,
                           kind="ExternalOutput").ap()

    scale = 1.0 / float(np.sqrt(D))

    with tile.TileContext(nc) as tc:
        with tc.tile_pool(name="io", bufs=3) as io_pool, \
             tc.tile_pool(name="vv", bufs=3) as v_pool, \
             tc.tile_pool(name="ee", bufs=3) as e_pool, \
             tc.tile_pool(name="e2", bufs=3) as e2_pool, \
             tc.tile_pool(name="rr", bufs=2) as r_pool, \
             tc.tile_pool(name="oo", bufs=2) as o_pool, \
             tc.tile_pool(name="cs", bufs=1) as c_pool, \
             tc.tile_pool(name="ps_sc", bufs=2, space="PSUM") as ps_sc, \
             tc.tile_pool(name="ps_d", bufs=2, space="PSUM") as ps_d, \
             tc.tile_pool(name="ps_pv", bufs=4, space="PSUM") as ps_pv:

            # Constant block-diagonal ones [128=(j,k), 128=(j,r)] used to
            # compute softmax denominators via one matmul per group.
            ones_bd = c_pool.tile([128, 128], dt.bfloat16, name="ones_bd")
            nc.vector.memset(ones_bd[:, :], 0.0)
            for j in range(4):
                nc.vector.memset(
                    ones_bd[32 * j:32 * j + 32, 32 * j:32 * j + 32], 1.0)

            qstate = {}
            gstate = {}

            def emit_loads(g):
                QT = io_pool.tile([128, 2, W, 32], dt.bfloat16, name="QT")
                KT = io_pool.tile([128, 2, W, 32], dt.bfloat16, name="KT")
                V4 = v_pool.tile([128, W, D], dt.bfloat16, name="V4")
                for p in range(2):
                    nc.sync.dma_start(
                        out=QT[:, p, :, :],
                        in_=qt_in[g, p].rearrange("pt (w q) -> pt w q",
                                                  w=W))
                    nc.sync.dma_start(
                        out=KT[:, p, :, :],
                        in_=kt_in[g, p].rearrange("pt (w q) -> pt w q",
                                                  w=W))
                nc.sync.dma_start(
                    out=V4[:, :, :],
                    in_=v_in[g].rearrange("pt (w d) -> pt w d", w=W))
                out_sb = o_pool.tile([128, W, D], dt.bfloat16, name="out_sb")
                qstate[g] = dict(QT=QT, KT=KT, V4=V4, out_sb=out_sb)

            def emit_scores(i):
                g, grp = divmod(i, 2)
                qs = qstate[g]
                QT, KT = qs["QT"], qs["KT"]
                psS = ps_sc.tile([128, 16, 32], dt.float32, name="psS")
                for wl in range(16):
                    w = 16 * grp + wl
                    for j in range(4):
                        p, jj = divmod(j, 2)
                        nc.tensor.matmul(
                            psS[32 * j:32 * j + 32, wl, :],
                            lhsT=KT[64 * jj:64 * jj + 64, p, w, :],
                            rhs=QT[64 * jj:64 * jj + 64, p, w, :],
                            start=True, stop=True,
                            tile_position=(64 * jj, 32 * j))
                E = e_pool.tile([128, 16, 32], dt.bfloat16, name="E")
                nc.scalar.activation(
                    E[:, :, :], psS[:, :, :],
                    mybir.ActivationFunctionType.Exp, scale=scale)
                gstate[i] = dict(E=E)

            def emit_denom(i):
                gs = gstate[i]
                E = gs["E"]
                psD = ps_d.tile([128, 16, 32], dt.float32, name="psD")
                nc.tensor.matmul(
                    psD[:, :, :], lhsT=ones_bd[:, :], rhs=E[:, :, :],
                    start=True, stop=True)
                RD = r_pool.tile([128, 16, 32], dt.float32, name="RD")
                nc.vector.reciprocal(RD[:, :, :], psD[:, :, :])
                E2 = e2_pool.tile([128, 16, 32], dt.bfloat16, name="E2")
                nc.vector.tensor_mul(E2[:, :, :], E[:, :, :], RD[:, :, :])
                gs["E2"] = E2

            def emit_pv(i):
                g, grp = divmod(i, 2)
                qs = qstate[g]
                gs = gstate.pop(i)
                V4, out_sb = qs["V4"], qs["out_sb"]
                E2 = gs["E2"]
                for half in range(2):
                    psPV = ps_pv.tile([128, 8, D], dt.float32, name="psPV")
                    for wl8 in range(8):
                        wl = 8 * half + wl8
                        w = 16 * grp + wl
                        for j in range(4):
                            nc.tensor.matmul(
                                psPV[32 * j:32 * j + 32, wl8, :],
                                lhsT=E2[32 * j:32 * j + 32, wl, :],
                                rhs=V4[32 * j:32 * j + 32, w, :],
                                start=True, stop=True,
                                tile_position=(32 * j, 32 * j))
                    w0 = 16 * grp + 8 * half
                    if half == 0:
                        nc.scalar.copy(out_sb[:, w0:w0 + 8, :],
                                       psPV[:, :, :])
                    else:
                        nc.vector.tensor_copy(out_sb[:, w0:w0 + 8, :],
                                              psPV[:, :, :])
                if grp == 1:
                    nc.sync.dma_start(
                        out=o_out[g].rearrange("pt (w d) -> pt w d", w=W),
                        in_=out_sb[:, :, :])
                    qstate.pop(g)

            # Software pipeline with a 2-group lag so the PE queue always
            # has runnable matmuls while exp / recip / normalize drain on
            # the Scalar and Vector engines.
            emit_loads(0)
            for i in range(NGRP):
                g, grp = divmod(i, 2)
                if grp == 0 and g + 1 < NQUAD:
                    emit_loads(g + 1)
                emit_scores(i)
                if i >= 1:
                    emit_denom(i - 1)
                if i >= 2:
                    emit_pv(i - 2)
            emit_denom(NGRP - 1)
            emit_pv(NGRP - 2)
            emit_pv(NGRP - 1)
    nc.compile()
    return nc


def _get_nc():
    global _CACHED_NC
    if _CACHED_NC is None:
        _CACHED_NC = _build_nc()
    return _CACHED_NC


def kernel(q, k, v, decode_step=0, decode_idx=0, _trace=False):
    from concourse.bass_utils import run_bass_kernel_spmd

    import ml_dtypes
    bf16 = ml_dtypes.bfloat16
    q = np.asarray(q, dtype=np.float32).reshape(NSLAB, H, W, D).astype(bf16)
    k = np.asarray(k, dtype=np.float32).reshape(NSLAB, H, W, D).astype(bf16)
    v = np.asarray(v, dtype=np.float32).reshape(NSLAB, H, W, D).astype(bf16)

    # d-major transpose for Q/K: (slab, d, w, h); V stays natural.
    qt = np.ascontiguousarray(q.transpose(0, 3, 2, 1))
    kt = np.ascontiguousarray(k.transpose(0, 3, 2, 1))

    nc = _get_nc()
    in_maps = []
    for c in range(N_CORES):
        sl = slice(c * NSLAB_CORE, (c + 1) * NSLAB_CORE)
        # (64, 64, 32, 32) -> (quad, pair, (jj,d), (w,q)) -> [16, 2, 128, 1024]
        qtc = qt[sl].reshape(NQUAD, 2, 2 * D, W * 32)
        ktc = kt[sl].reshape(NQUAD, 2, 2 * D, W * 32)
        vc = v[sl].reshape(NQUAD, 128, W * D)
        in_maps.append({
            "qt_in": np.ascontiguousarray(qtc),
            "kt_in": np.ascontiguousarray(ktc),
            "v_in": np.ascontiguousarray(vc),
        })
    res = run_bass_kernel_spmd(nc, in_maps, core_ids=list(range(N_CORES)),
                               trace=_trace)
    outs = []
    for r in res.results:
        # [16, 128, 2048] = (quad, (j, h), (w, d)) -> (slab, h, w, d)
        o = np.asarray(r["o_out"]).reshape(NSLAB_CORE, H, W, D)
        outs.append(o)
    out = np.concatenate(outs, axis=0).astype(np.float32)
    out = out.reshape(B, NH, T, H, W, D)
    if _trace:
        return out, res
    return out


if __name__ == "__main__":
    rng = np.random.default_rng(0)
    shape = (B, NH, T, H, W, D)
    q = rng.standard_normal(shape, dtype=np.float32)
    k = rng.standard_normal(shape, dtype=np.float32)
    v = rng.standard_normal(shape, dtype=np.float32)
    out = kernel(q, k, v)
    print("kernel ran, out shape", out.shape)


# revision 6
# speedup vs baseline: 1.4478x; 1.2444x over previous
"""AxialAttention Trainium2 Bass kernel (v2).

Problem: q,k,v of shape (4, 8, 16, 32, 32, 64) = (b, heads, t, h, w, d),
attention along the h axis (axis 3), softmax over keys, out same shape.

The computation is 512 independent "slabs" (b, heads, t), each a batch of
w=32 independent length-32 attention problems with head dim 64.  64 slabs
per NeuronCore (8 cores), processed in "quads" (4 slabs = 128 partitions).

Design notes (all matmul cost on the PE is ~ output-free-size columns, so
outputs are kept 128 partitions wide and as narrow as possible in free):

  - Host pre-transposes Q and K to d-major layout, so no on-chip
    transposes are needed and every DMA is fully contiguous.
  - Scores: per (slab j, w) one K=64 matmul; the four slabs of a quad are
    packed as two "pair" tiles [128=(jj,d64), ...] and placed at PE
    quadrants (64*jj, 32*j), producing psS [128=(j,k), (w,q)] in PSUM.
  - exp on ScalarE over [128, 512] tiles (scale = 1/sqrt(64)).
  - Softmax denominator: one N=512 matmul per 16-w group with a constant
    block-diagonal ones matrix as weights: psD[(j,r),(w,q)] = sum_k
    E[(j,k),(w,q)].  Since every partition of a band carries the same
    value, RD = 1/psD aligns with E partition-for-partition, and
    E2 = E * RD normalizes E before PV (DVE).
  - PV: per (slab j, w) one K=32 matmul lhsT=E2 block, rhs=V natural
    [128=(j,k), (w,d)], at PE quadrant (32j, 32j) -> psPV [128=(j,q), d].
  - psPV copied (and cast) to bf16 out_sb, stored contiguously; host
    casts back to fp32.
"""

import os
import sys
import numpy as np

for _p in ("/root/.axon_site/_ro/trn_rl_repo", "/opt/trn_rl_repo"):
    if os.path.isdir(_p) and _p not in sys.path:
        sys.path.append(_p)

B, NH, T, H, W, D = 4, 8, 16, 32, 32, 64
N_CORES = 8
NSLAB = B * NH * T  # 512
NSLAB_CORE = NSLAB // N_CORES  # 64
NQUAD = NSLAB_CORE // 4  # 16
NGRP = 2 * NQUAD  # 16-w score/exp/pv groups per core

_CACHED_NC = None


def _build_nc():
    import concourse.bacc as bacc
    import concourse.mybir as mybir
    from concourse import tile

    dt = mybir.dt

    nc = bacc.Bacc("TRN2", target_bir_lowering=False, debug=False,
                   num_devices=N_CORES)
    # (quad, pair, (jj,d64), (w,q))
    qt_in = nc.dram_tensor("qt_in", [NQUAD, 2, 128, W * 32], dt.bfloat16,
                           kind="ExternalInput").ap()
    kt_in = nc.dram_tensor("kt_in", [NQUAD, 2, 128, W * 32], dt.bfloat16,
                           kind="ExternalInput").ap()
    # (quad, (j,k=h), (w,d))
    v_in = nc.dram_tensor("v_in", [NQUAD, 128, W * D], dt.bfloat16,
                          kind="ExternalInput").ap()
    # (quad, (j,q=h), (w,d))
    o_out = nc.dram_tensor("o_out", [NQUAD, 128, W * D], dt.bfloat16,
                           kind="ExternalOutput").ap()

    scale = 1.0 / float(np.sqrt(D))

    with tile.TileContext(nc) as tc:
        with tc.tile_pool(name="io", bufs=3) as io_pool, \
             tc.tile_pool(name="vv", bufs=3) as v_pool, \
             tc.tile_pool(name="ee", bufs=3) as e_pool, \
             tc.tile_pool(name="e2", bufs=3) as e2_pool, \
             tc.tile_pool(name="rr", bufs=2) as r_pool, \
             tc.tile_pool(name="oo", bufs=2) as o_pool, \
             tc.tile_pool(name="cs", bufs=1) as c_pool, \
             tc.tile_pool(name="ps_sc", bufs=2, space="PSUM") as ps_sc, \
             tc.tile_pool(name="ps_d", bufs=2, space="PSUM") as ps_d, \
             tc.tile_pool(name="ps_pv", bufs=4, space="PSUM") as ps_pv:

            # Constant block-diagonal ones [128=(j,k), 128=(j,r)] used to
            # compute softmax denominators via one matmul per group.
            ones_bd = c_pool.tile([128, 128], dt.bfloat16, name="ones_bd")
            nc.vector.memset(ones_bd[:, :], 0.0)
            for j in range(4):
                nc.vector.memset(
                    ones_bd[32 * j:32 * j + 32, 32 * j:32 * j + 32], 1.0)

            qstate = {}
            gstate = {}

            def emit_loads(g):
                QT = io_pool.tile([128, 2, W, 32], dt.bfloat16, name="QT")
                KT = io_pool.tile([128, 2, W, 32], dt.bfloat16, name="KT")
                V4 = v_pool.tile([128, W, D], dt.bfloat16, name="V4")
                for p in range(2):
                    nc.sync.dma_start(
                        out=QT[:, p, :, :],
                        in_=qt_in[g, p].rearrange("pt (w q) -> pt w q",
                                                  w=W))
                    nc.sync.dma_start(
                        out=KT[:, p, :, :],
                        in_=kt_in[g, p].rearrange("pt (w q) -> pt w q",
                                                  w=W))
                nc.sync.dma_start(
                    out=V4[:, :, :],
                    in_=v_in[g].rearrange("pt (w d) -> pt w d", w=W))
                out_sb = o_pool.tile([128, W, D], dt.bfloat16, name="out_sb")
                qstate[g] = dict(QT=QT, KT=KT, V4=V4, out_sb=out_sb)

            def emit_scores(i):
                g, grp = divmod(i, 2)
                qs = qstate[g]
                QT, KT = qs["QT"], qs["KT"]
                psS = ps_sc.tile([128, 16, 32], dt.float32, name="psS")
                for wl in range(16):
                    w = 16 * grp + wl
                    for j in range(4):
                        p, jj = divmod(j, 2)
                        nc.tensor.matmul(
                            psS[32 * j:32 * j + 32, wl, :],
                            lhsT=KT[64 * jj:64 * jj + 64, p, w, :],
                            rhs=QT[64 * jj:64 * jj + 64, p, w, :],
                            start=True, stop=True,
                            tile_position=(64 * jj, 32 * j))
                E = e_pool.tile([128, 16, 32], dt.bfloat16, name="E")
                nc.scalar.activation(
                    E[:, :, :], psS[:, :, :],
                    mybir.ActivationFunctionType.Exp, scale=scale)
                gstate[i] = dict(E=E)

            def emit_denom(i):
                gs = gstate[i]
                E = gs["E"]
                psD = ps_d.tile([128, 16, 32], dt.float32, name="psD")
                nc.tensor.matmul(
                    psD[:, :, :], lhsT=ones_bd[:, :], rhs=E[:, :, :],
                    start=True, stop=True)
                RD = r_pool.tile([128, 16, 32], dt.float32, name="RD")
                nc.vector.reciprocal_approx_fast(out=RD[:, :, :],
                                                 in_=psD[:, :, :])
                E2 = e2_pool.tile([128, 16, 32], dt.bfloat16, name="E2")
                nc.vector.tensor_mul(E2[:, :, :], E[:, :, :], RD[:, :, :])
                gs["E2"] = E2

            def emit_pv(i):
                g, grp = divmod(i, 2)
                qs = qstate[g]
                gs = gstate.pop(i)
                V4, out_sb = qs["V4"], qs["out_sb"]
                E2 = gs["E2"]
                for half in range(2):
                    psPV = ps_pv.tile([128, 8, D], dt.float32, name="psPV")
                    for wl8 in range(8):
                        wl = 8 * half + wl8
                        w = 16 * grp + wl
                        for j in range(4):
                            nc.tensor.matmul(
                                psPV[32 * j:32 * j + 32, wl8, :],
                                lhsT=E2[32 * j:32 * j + 32, wl, :],
                                rhs=V4[32 * j:32 * j + 32, w, :],
                                start=True, stop=True,
                                tile_position=(32 * j, 32 * j))
                    w0 = 16 * grp + 8 * half
                    if half == 0:
                        nc.scalar.copy(out_sb[:, w0:w0 + 8, :],
                                       psPV[:, :, :])
                    else:
                        nc.vector.tensor_copy(out_sb[:, w0:w0 + 8, :],
                                              psPV[:, :, :])
                if grp == 1:
                    nc.gpsimd.dma_start(
                        out=o_out[g].rearrange("pt (w d) -> pt w d", w=W),
                        in_=out_sb[:, :, :])
                    qstate.pop(g)

            # Software pipeline with a 2-group lag so the PE queue always
            # has runnable matmuls while exp / recip / normalize drain on
            # the Scalar and Vector engines.
            emit_loads(0)
            for i in range(NGRP):
                g, grp = divmod(i, 2)
                if grp == 0 and g + 1 < NQUAD:
                    emit_loads(g + 1)
                emit_scores(i)
                if i >= 1:
                    emit_denom(i - 1)
                if i >= 2:
                    emit_pv(i - 2)
            emit_denom(NGRP - 1)
            emit_pv(NGRP - 2)
            emit_pv(NGRP - 1)
    nc.compile()
    return nc


def _get_nc():
    global _CACHED_NC
    if _CACHED_NC is None:
        _CACHED_NC = _build_nc()
    return _CACHED_NC


def kernel(q, k, v, decode_step=0, decode_idx=0, _trace=False):
    from concourse.bass_utils import run_bass_kernel_spmd

    import ml_dtypes
    bf16 = ml_dtypes.bfloat16
    q = np.asarray(q, dtype=np.float32).reshape(NSLAB, H, W, D).astype(bf16)
    k = np.asarray(k, dtype=np.float32).reshape(NSLAB, H, W, D).astype(bf16)
    v = np.asarray(v, dtype=np.float32).reshape(NSLAB, H, W, D).astype(bf16)

    # d-major transpose for Q/K: (slab, d, w, h); V stays natural.
    qt = np.ascontiguousarray(q.transpose(0, 3, 2, 1))
    kt = np.ascontiguousarray(k.transpose(0, 3, 2, 1))

    nc = _get_nc()
    in_maps = []
    for c in range(N_CORES):
        sl = slice(c * NSLAB_CORE, (c + 1) * NSLAB_CORE)
        # (64, 64, 32, 32) -> (quad, pair, (jj,d), (w,q)) -> [16, 2, 128, 1024]
        qtc = qt[sl].reshape(NQUAD, 2, 2 * D, W * 32)
        ktc = kt[sl].reshape(NQUAD, 2, 2 * D, W * 32)
        vc = v[sl].reshape(NQUAD, 128, W * D)
        in_maps.append({
            "qt_in": np.ascontiguousarray(qtc),
            "kt_in": np.ascontiguousarray(ktc),
            "v_in": np.ascontiguousarray(vc),
        })
    res = run_bass_kernel_spmd(nc, in_maps, core_ids=list(range(N_CORES)),
                               trace=_trace)
    outs = []
    for r in res.results:
        # [16, 128, 2048] = (quad, (j, h), (w, d)) -> (slab, h, w, d)
        o = np.asarray(r["o_out"]).reshape(NSLAB_CORE, H, W, D)
        outs.append(o)
    out = np.concatenate(outs, axis=0).astype(np.float32)
    out = out.reshape(B, NH, T, H, W, D)
    if _trace:
        return out, res
    return out


if __name__ == "__main__":
    rng = np.random.default_rng(0)
    shape = (B, NH, T, H, W, D)
    q = rng.standard_normal(shape, dtype=np.float32)
    k = rng.standard_normal(shape, dtype=np.float32)
    v = rng.standard_normal(shape, dtype=np.float32)
    out = kernel(q, k, v)
    print("kernel ran, out shape", out.shape)


# revision 9
# speedup vs baseline: 1.8600x; 1.2847x over previous
"""AxialAttention Trainium2 Bass kernel (v2).

Problem: q,k,v of shape (4, 8, 16, 32, 32, 64) = (b, heads, t, h, w, d),
attention along the h axis (axis 3), softmax over keys, out same shape.

The computation is 512 independent "slabs" (b, heads, t), each a batch of
w=32 independent length-32 attention problems with head dim 64.  64 slabs
per NeuronCore (8 cores), processed in "quads" (4 slabs = 128 partitions).

Design notes (all matmul cost on the PE is ~ output-free-size columns, so
outputs are kept 128 partitions wide and as narrow as possible in free):

  - Host pre-transposes Q and K to d-major layout, so no on-chip
    transposes are needed and every DMA is fully contiguous.
  - Scores: per (slab j, w) one K=64 matmul; the four slabs of a quad are
    packed as two "pair" tiles [128=(jj,d64), ...] and placed at PE
    quadrants (64*jj, 32*j), producing psS [128=(j,k), (w,q)] in PSUM.
  - exp on ScalarE over [128, 512] tiles (scale = 1/sqrt(64)).
  - Softmax denominator: one N=512 matmul per 16-w group with a constant
    block-diagonal ones matrix as weights: psD[(j,r),(w,q)] = sum_k
    E[(j,k),(w,q)].  Since every partition of a band carries the same
    value, RD = 1/psD aligns with E partition-for-partition, and
    E2 = E * RD normalizes E before PV (DVE).
  - PV: per (slab j, w) one K=32 matmul lhsT=E2 block, rhs=V natural
    [128=(j,k), (w,d)], at PE quadrant (32j, 32j) -> psPV [128=(j,q), d].
  - psPV copied (and cast) to bf16 out_sb, stored contiguously; host
    casts back to fp32.
"""

import os
import sys
import numpy as np

for _p in ("/root/.axon_site/_ro/trn_rl_repo", "/opt/trn_rl_repo"):
    if os.path.isdir(_p) and _p not in sys.path:
        sys.path.append(_p)

B, NH, T, H, W, D = 4, 8, 16, 32, 32, 64
N_CORES = 8
NSLAB = B * NH * T  # 512
NSLAB_CORE = NSLAB // N_CORES  # 64
NQUAD = NSLAB_CORE // 4  # 16
NGRP = 2 * NQUAD  # 16-w score/exp/pv groups per core

_CACHED_NC = None


def _build_nc():
    import concourse.bacc as bacc
    import concourse.mybir as mybir
    from concourse import tile

    dt = mybir.dt

    nc = bacc.Bacc("TRN2", target_bir_lowering=False, debug=False,
                   num_devices=N_CORES)
    # (quad, pair, (jj,d64), (w,q))
    qt_in = nc.dram_tensor("qt_in", [NQUAD, 2, 128, W * 32], dt.bfloat16,
                           kind="ExternalInput").ap()
    kt_in = nc.dram_tensor("kt_in", [NQUAD, 2, 128, W * 32], dt.bfloat16,
                           kind="ExternalInput").ap()
    # (quad, (j,k=h), (w,d))
    v_in = nc.dram_tensor("v_in", [NQUAD, 128, W * D], dt.bfloat16,
                          kind="ExternalInput").ap()
    # (quad, (j,q=h), (w,d))
    o_out = nc.dram_tensor("o_out", [NQUAD, 128, W * D], dt.bfloat16,
                           kind="ExternalOutput").ap()

    scale = 1.0 / float(np.sqrt(D))

    with tile.TileContext(nc) as tc:
        with tc.tile_pool(name="io", bufs=3) as io_pool, \
             tc.tile_pool(name="vv", bufs=3) as v_pool, \
             tc.tile_pool(name="ee", bufs=3) as e_pool, \
             tc.tile_pool(name="e2", bufs=3) as e2_pool, \
             tc.tile_pool(name="rr", bufs=2) as r_pool, \
             tc.tile_pool(name="oo", bufs=2) as o_pool, \
             tc.tile_pool(name="cs", bufs=1) as c_pool, \
             tc.tile_pool(name="ps_sc", bufs=2, space="PSUM") as ps_sc, \
             tc.tile_pool(name="ps_d", bufs=2, space="PSUM") as ps_d, \
             tc.tile_pool(name="ps_pv", bufs=4, space="PSUM") as ps_pv:

            # Constant block-diagonal ones [128=(j,k), 128=(j,r)] used to
            # compute softmax denominators via one matmul per group.
            ones_bd = c_pool.tile([128, 128], dt.bfloat16, name="ones_bd")
            nc.vector.memset(ones_bd[:, :], 0.0)
            for j in range(4):
                nc.vector.memset(
                    ones_bd[32 * j:32 * j + 32, 32 * j:32 * j + 32], 1.0)

            qstate = {}
            gstate = {}

            def emit_loads(g):
                QT = io_pool.tile([128, 2, W, 32], dt.bfloat16, name="QT")
                KT = io_pool.tile([128, 2, W, 32], dt.bfloat16, name="KT")
                V4 = v_pool.tile([128, W, D], dt.bfloat16, name="V4")
                for p in range(2):
                    nc.sync.dma_start(
                        out=QT[:, p, :, :],
                        in_=qt_in[g, p].rearrange("pt (w q) -> pt w q",
                                                  w=W))
                    nc.sync.dma_start(
                        out=KT[:, p, :, :],
                        in_=kt_in[g, p].rearrange("pt (w q) -> pt w q",
                                                  w=W))
                nc.sync.dma_start(
                    out=V4[:, :, :],
                    in_=v_in[g].rearrange("pt (w d) -> pt w d", w=W))
                out_sb = o_pool.tile([128, W, D], dt.bfloat16, name="out_sb")
                qstate[g] = dict(QT=QT, KT=KT, V4=V4, out_sb=out_sb)

            def emit_scores(i):
                g, grp = divmod(i, 2)
                qs = qstate[g]
                QT, KT = qs["QT"], qs["KT"]
                psS = ps_sc.tile([128, 16, 32], dt.float32, name="psS")
                for wl in range(16):
                    w = 16 * grp + wl
                    for j in range(4):
                        p, jj = divmod(j, 2)
                        nc.tensor.matmul(
                            psS[32 * j:32 * j + 32, wl, :],
                            lhsT=KT[64 * jj:64 * jj + 64, p, w, :],
                            rhs=QT[64 * jj:64 * jj + 64, p, w, :],
                            start=True, stop=True,
                            tile_position=(64 * jj, 32 * j))
                E = e_pool.tile([128, 16, 32], dt.bfloat16, name="E")
                nc.scalar.activation(
                    E[:, :, :], psS[:, :, :],
                    mybir.ActivationFunctionType.Exp, scale=scale)
                gstate[i] = dict(E=E)

            e2bd_allocs = [0]

            def emit_denom(i):
                gs = gstate[i]
                E = gs["E"]
                psD = ps_d.tile([128, 16, 32], dt.float32, name="psD")
                nc.tensor.matmul(
                    psD[:, :, :], lhsT=ones_bd[:, :], rhs=E[:, :, :],
                    start=True, stop=True)
                RD = r_pool.tile([128, 16, 32], dt.float32, name="RD")
                nc.vector.reciprocal_approx_fast(out=RD[:, :, :],
                                                 in_=psD[:, :, :])
                RDb = r_pool.tile([128, 16, 32], dt.bfloat16, name="RDb")
                nc.vector.tensor_copy(RDb[:, :, :], RD[:, :, :])
                # Normalized E scattered into a block-diagonal weight tile
                # [128=(j,k), (w, j', q)]; off-diagonal blocks stay zero
                # (zeroed once per pool buffer).
                E2bd = e2_pool.tile([128, 16, 4, 32], dt.bfloat16,
                                    name="E2bd")
                if e2bd_allocs[0] < 3:
                    e2bd_allocs[0] += 1
                    nc.vector.memset(E2bd[:, :, :, :], 0.0)
                for j in range(4):
                    nc.vector.tensor_mul(
                        E2bd[32 * j:32 * j + 32, :, j, :],
                        E[32 * j:32 * j + 32, :, :],
                        RDb[32 * j:32 * j + 32, :, :])
                gs["E2bd"] = E2bd

            def emit_pv(i):
                g, grp = divmod(i, 2)
                qs = qstate[g]
                gs = gstate.pop(i)
                V4, out_sb = qs["V4"], qs["out_sb"]
                E2bd = gs["E2bd"]
                for half in range(2):
                    psPV = ps_pv.tile([128, 8, D], dt.float32, name="psPV")
                    for wl8 in range(8):
                        wl = 8 * half + wl8
                        w = 16 * grp + wl
                        nc.tensor.matmul(
                            psPV[:, wl8, :],
                            lhsT=E2bd[:, wl, :, :],
                            rhs=V4[:, w, :],
                            start=True, stop=True)
                    w0 = 16 * grp + 8 * half
                    nc.scalar.copy(out_sb[:, w0:w0 + 8, :], psPV[:, :, :])
                if grp == 1:
                    nc.gpsimd.dma_start(
                        out=o_out[g].rearrange("pt (w d) -> pt w d", w=W),
                        in_=out_sb[:, :, :])
                    qstate.pop(g)

            # Software pipeline with a 2-group lag so the PE queue always
            # has runnable matmuls while exp / recip / normalize drain on
            # the Scalar and Vector engines.
            emit_loads(0)
            for i in range(NGRP):
                g, grp = divmod(i, 2)
                if grp == 0 and g + 1 < NQUAD:
                    emit_loads(g + 1)
                emit_scores(i)
                if i >= 1:
                    emit_denom(i - 1)
                if i >= 2:
                    emit_pv(i - 2)
            emit_denom(NGRP - 1)
            emit_pv(NGRP - 2)
            emit_pv(NGRP - 1)
    nc.compile()
    return nc


def _get_nc():
    global _CACHED_NC
    if _CACHED_NC is None:
        _CACHED_NC = _build_nc()
    return _CACHED_NC


def kernel(q, k, v, decode_step=0, decode_idx=0, _trace=False):
    from concourse.bass_utils import run_bass_kernel_spmd

    import ml_dtypes
    bf16 = ml_dtypes.bfloat16
    q = np.asarray(q, dtype=np.float32).reshape(NSLAB, H, W, D).astype(bf16)
    k = np.asarray(k, dtype=np.float32).reshape(NSLAB, H, W, D).astype(bf16)
    v = np.asarray(v, dtype=np.float32).reshape(NSLAB, H, W, D).astype(bf16)

    # d-major transpose for Q/K: (slab, d, w, h); V stays natural.
    qt = np.ascontiguousarray(q.transpose(0, 3, 2, 1))
    kt = np.ascontiguousarray(k.transpose(0, 3, 2, 1))

    nc = _get_nc()
    in_maps = []
    for c in range(N_CORES):
        sl = slice(c * NSLAB_CORE, (c + 1) * NSLAB_CORE)
        # (64, 64, 32, 32) -> (quad, pair, (jj,d), (w,q)) -> [16, 2, 128, 1024]
        qtc = qt[sl].reshape(NQUAD, 2, 2 * D, W * 32)
        ktc = kt[sl].reshape(NQUAD, 2, 2 * D, W * 32)
        vc = v[sl].reshape(NQUAD, 128, W * D)
        in_maps.append({
            "qt_in": np.ascontiguousarray(qtc),
            "kt_in": np.ascontiguousarray(ktc),
            "v_in": np.ascontiguousarray(vc),
        })
    res = run_bass_kernel_spmd(nc, in_maps, core_ids=list(range(N_CORES)),
                               trace=_trace)
    outs = []
    for r in res.results:
        # [16, 128, 2048] = (quad, (j, h), (w, d)) -> (slab, h, w, d)
        o = np.asarray(r["o_out"]).reshape(NSLAB_CORE, H, W, D)
        outs.append(o)
    out = np.concatenate(outs, axis=0).astype(np.float32)
    out = out.reshape(B, NH, T, H, W, D)
    if _trace:
        return out, res
    return out


if __name__ == "__main__":
    rng = np.random.default_rng(0)
    shape = (B, NH, T, H, W, D)
    q = rng.standard_normal(shape, dtype=np.float32)
    k = rng.standard_normal(shape, dtype=np.float32)
    v = rng.standard_normal(shape, dtype=np.float32)
    out = kernel(q, k, v)
    print("kernel ran, out shape", out.shape)


# revision 14
# speedup vs baseline: 1.8848x; 1.0133x over previous
"""AxialAttention Trainium2 Bass kernel (v2).

Problem: q,k,v of shape (4, 8, 16, 32, 32, 64) = (b, heads, t, h, w, d),
attention along the h axis (axis 3), softmax over keys, out same shape.

The computation is 512 independent "slabs" (b, heads, t), each a batch of
w=32 independent length-32 attention problems with head dim 64.  64 slabs
per NeuronCore (8 cores), processed in "quads" (4 slabs = 128 partitions).

Design notes (all matmul cost on the PE is ~ output-free-size columns, so
outputs are kept 128 partitions wide and as narrow as possible in free):

  - Host pre-transposes Q and K to d-major layout, so no on-chip
    transposes are needed and every DMA is fully contiguous.
  - Scores: per (slab j, w) one K=64 matmul; the four slabs of a quad are
    packed as two "pair" tiles [128=(jj,d64), ...] and placed at PE
    quadrants (64*jj, 32*j), producing psS [128=(j,k), (w,q)] in PSUM.
  - exp on ScalarE over [128, 512] tiles (scale = 1/sqrt(64)).
  - Softmax denominator: one N=512 matmul per 16-w group with a constant
    block-diagonal ones matrix as weights: psD[(j,r),(w,q)] = sum_k
    E[(j,k),(w,q)].  Since every partition of a band carries the same
    value, RD = 1/psD aligns with E partition-for-partition, and
    E2 = E * RD normalizes E before PV (DVE).
  - PV: per (slab j, w) one K=32 matmul lhsT=E2 block, rhs=V natural
    [128=(j,k), (w,d)], at PE quadrant (32j, 32j) -> psPV [128=(j,q), d].
  - psPV copied (and cast) to bf16 out_sb, stored contiguously; host
    casts back to fp32.
"""

import os
import sys
import numpy as np

for _p in ("/root/.axon_site/_ro/trn_rl_repo", "/opt/trn_rl_repo"):
    if os.path.isdir(_p) and _p not in sys.path:
        sys.path.append(_p)

B, NH, T, H, W, D = 4, 8, 16, 32, 32, 64
N_CORES = 8
NSLAB = B * NH * T  # 512
NSLAB_CORE = NSLAB // N_CORES  # 64
NQUAD = NSLAB_CORE // 4  # 16
NGRP = 2 * NQUAD  # 16-w score/exp/pv groups per core

_CACHED_NC = None


def _build_nc():
    import concourse.bacc as bacc
    import concourse.mybir as mybir
    from concourse import tile

    dt = mybir.dt

    nc = bacc.Bacc("TRN2", target_bir_lowering=False, debug=False,
                   num_devices=N_CORES)
    # (quad, pair, (jj,d64), (w,q))
    qt_in = nc.dram_tensor("qt_in", [NQUAD, 2, 128, W * 32], dt.bfloat16,
                           kind="ExternalInput").ap()
    kt_in = nc.dram_tensor("kt_in", [NQUAD, 2, 128, W * 32], dt.bfloat16,
                           kind="ExternalInput").ap()
    # (quad, (j,k=h), (w,d))
    v_in = nc.dram_tensor("v_in", [NQUAD, 128, W * D], dt.bfloat16,
                          kind="ExternalInput").ap()
    # (quad, (j,q=h), (w,d))
    o_out = nc.dram_tensor("o_out", [NQUAD, 128, W * D], dt.bfloat16,
                           kind="ExternalOutput").ap()

    scale = 1.0 / float(np.sqrt(D))

    with tile.TileContext(nc) as tc:
        with tc.tile_pool(name="io", bufs=4) as io_pool, \
             tc.tile_pool(name="vv", bufs=4) as v_pool, \
             tc.tile_pool(name="ee", bufs=3) as e_pool, \
             tc.tile_pool(name="e2", bufs=3) as e2_pool, \
             tc.tile_pool(name="rr", bufs=2) as r_pool, \
             tc.tile_pool(name="oo", bufs=3) as o_pool, \
             tc.tile_pool(name="cs", bufs=1) as c_pool, \
             tc.tile_pool(name="ps_sc", bufs=2, space="PSUM") as ps_sc, \
             tc.tile_pool(name="ps_d", bufs=2, space="PSUM") as ps_d, \
             tc.tile_pool(name="ps_pv", bufs=4, space="PSUM") as ps_pv:

            # Constant block-diagonal ones [128=(j,k), 128=(j,r)] used to
            # compute softmax denominators via one matmul per group.
            ones_bd = c_pool.tile([128, 128], dt.bfloat16, name="ones_bd")
            nc.vector.memset(ones_bd[:, :], 0.0)
            for j in range(4):
                nc.vector.memset(
                    ones_bd[32 * j:32 * j + 32, 32 * j:32 * j + 32], 1.0)

            qstate = {}
            gstate = {}

            def emit_loads(g):
                QT = io_pool.tile([128, 2, W, 32], dt.bfloat16, name="QT")
                KT = io_pool.tile([128, 2, W, 32], dt.bfloat16, name="KT")
                V4 = v_pool.tile([128, W, D], dt.bfloat16, name="V4")
                for p in range(2):
                    nc.sync.dma_start(
                        out=QT[:, p, :, :],
                        in_=qt_in[g, p].rearrange("pt (w q) -> pt w q",
                                                  w=W))
                    nc.sync.dma_start(
                        out=KT[:, p, :, :],
                        in_=kt_in[g, p].rearrange("pt (w q) -> pt w q",
                                                  w=W))
                nc.sync.dma_start(
                    out=V4[:, :, :],
                    in_=v_in[g].rearrange("pt (w d) -> pt w d", w=W))
                out_sb = o_pool.tile([128, W, D], dt.bfloat16, name="out_sb")
                qstate[g] = dict(QT=QT, KT=KT, V4=V4, out_sb=out_sb)

            def emit_scores(i):
                g, grp = divmod(i, 2)
                qs = qstate[g]
                QT, KT = qs["QT"], qs["KT"]
                psS = ps_sc.tile([128, 16, 32], dt.float32, name="psS")
                for wl in range(16):
                    w = 16 * grp + wl
                    for j in range(4):
                        p, jj = divmod(j, 2)
                        nc.tensor.matmul(
                            psS[32 * j:32 * j + 32, wl, :],
                            lhsT=KT[64 * jj:64 * jj + 64, p, w, :],
                            rhs=QT[64 * jj:64 * jj + 64, p, w, :],
                            start=True, stop=True,
                            tile_position=(64 * jj, 32 * j))
                E = e_pool.tile([128, 16, 32], dt.bfloat16, name="E")
                nc.scalar.activation(
                    E[:, :, :], psS[:, :, :],
                    mybir.ActivationFunctionType.Exp, scale=scale)
                gstate[i] = dict(E=E)

            e2bd_allocs = [0]

            def emit_denom(i):
                gs = gstate[i]
                E = gs["E"]
                psD = ps_d.tile([128, 16, 32], dt.float32, name="psD")
                nc.tensor.matmul(
                    psD[:, :, :], lhsT=ones_bd[:, :], rhs=E[:, :, :],
                    start=True, stop=True)
                RD = r_pool.tile([128, 16, 32], dt.float32, name="RD")
                nc.vector.reciprocal_approx_fast(out=RD[:, :, :],
                                                 in_=psD[:, :, :])
                RDb = r_pool.tile([128, 16, 32], dt.bfloat16, name="RDb")
                nc.vector.tensor_copy(RDb[:, :, :], RD[:, :, :])
                # Normalized E scattered into a block-diagonal weight tile
                # [128=(j,k), (w, j', q)]; off-diagonal blocks stay zero
                # (zeroed once per pool buffer).
                E2bd = e2_pool.tile([128, 16, 4, 32], dt.bfloat16,
                                    name="E2bd")
                if e2bd_allocs[0] < 3:
                    e2bd_allocs[0] += 1
                    nc.vector.memset(E2bd[:, :, :, :], 0.0)
                for j in range(4):
                    nc.vector.tensor_mul(
                        E2bd[32 * j:32 * j + 32, :, j, :],
                        E[32 * j:32 * j + 32, :, :],
                        RDb[32 * j:32 * j + 32, :, :])
                gs["E2bd"] = E2bd

            def emit_pv(i):
                g, grp = divmod(i, 2)
                qs = qstate[g]
                gs = gstate.pop(i)
                V4, out_sb = qs["V4"], qs["out_sb"]
                E2bd = gs["E2bd"]
                for half in range(2):
                    psPV = ps_pv.tile([128, 8, D], dt.float32, name="psPV")
                    for wl8 in range(8):
                        wl = 8 * half + wl8
                        w = 16 * grp + wl
                        nc.tensor.matmul(
                            psPV[:, wl8, :],
                            lhsT=E2bd[:, wl, :, :],
                            rhs=V4[:, w, :],
                            start=True, stop=True)
                    w0 = 16 * grp + 8 * half
                    nc.scalar.copy(out_sb[:, w0:w0 + 8, :], psPV[:, :, :])
                if grp == 1:
                    nc.gpsimd.dma_start(
                        out=o_out[g].rearrange("pt (w d) -> pt w d", w=W),
                        in_=out_sb[:, :, :])
                    qstate.pop(g)

            # Software pipeline with a 2-group lag so the PE queue always
            # has runnable matmuls while exp / recip / normalize drain on
            # the Scalar and Vector engines.  PV/copies are emitted before
            # scores/exp each round so the Scalar queue frees PSUM buffers
            # before blocking on the next exp's input.
            emit_loads(0)
            emit_loads(1)
            for i in range(NGRP):
                g, grp = divmod(i, 2)
                if grp == 0 and g + 2 < NQUAD:
                    emit_loads(g + 2)
                if i >= 2:
                    emit_pv(i - 2)
                emit_scores(i)
                if i >= 1:
                    emit_denom(i - 1)
            emit_denom(NGRP - 1)
            emit_pv(NGRP - 2)
            emit_pv(NGRP - 1)
    nc.compile()
    return nc


def _get_nc():
    global _CACHED_NC
    if _CACHED_NC is None:
        _CACHED_NC = _build_nc()
    return _CACHED_NC


def kernel(q, k, v, decode_step=0, decode_idx=0, _trace=False):
    from concourse.bass_utils import run_bass_kernel_spmd

    import ml_dtypes
    bf16 = ml_dtypes.bfloat16
    q = np.asarray(q, dtype=np.float32).reshape(NSLAB, H, W, D).astype(bf16)
    k = np.asarray(k, dtype=np.float32).reshape(NSLAB, H, W, D).astype(bf16)
    v = np.asarray(v, dtype=np.float32).reshape(NSLAB, H, W, D).astype(bf16)

    # d-major transpose for Q/K: (slab, d, w, h); V stays natural.
    qt = np.ascontiguousarray(q.transpose(0, 3, 2, 1))
    kt = np.ascontiguousarray(k.transpose(0, 3, 2, 1))

    nc = _get_nc()
    in_maps = []
    for c in range(N_CORES):
        sl = slice(c * NSLAB_CORE, (c + 1) * NSLAB_CORE)
        # (64, 64, 32, 32) -> (quad, pair, (jj,d), (w,q)) -> [16, 2, 128, 1024]
        qtc = qt[sl].reshape(NQUAD, 2, 2 * D, W * 32)
        ktc = kt[sl].reshape(NQUAD, 2, 2 * D, W * 32)
        vc = v[sl].reshape(NQUAD, 128, W * D)
        in_maps.append({
            "qt_in": np.ascontiguousarray(qtc),
            "kt_in": np.ascontiguousarray(ktc),
            "v_in": np.ascontiguousarray(vc),
        })
    res = run_bass_kernel_spmd(nc, in_maps, core_ids=list(range(N_CORES)),
                               trace=_trace)
    outs = []
    for r in res.results:
        # [16, 128, 2048] = (quad, (j, h), (w, d)) -> (slab, h, w, d)
        o = np.asarray(r["o_out"]).reshape(NSLAB_CORE, H, W, D)
        outs.append(o)
    out = np.concatenate(outs, axis=0).astype(np.float32)
    out = out.reshape(B, NH, T, H, W, D)
    if _trace:
        return out, res
    return out


if __name__ == "__main__":
    rng = np.random.default_rng(0)
    shape = (B, NH, T, H, W, D)
    q = rng.standard_normal(shape, dtype=np.float32)
    k = rng.standard_normal(shape, dtype=np.float32)
    v = rng.standard_normal(shape, dtype=np.float32)
    out = kernel(q, k, v)
    print("kernel ran, out shape", out.shape)
